# revision 3
# baseline (speedup 1.0000x reference)
"""AMCNN attention Bass kernel for Trainium2, 8 NeuronCores, data-parallel over batch.

Reference computation (per channel c, example b, X = inputs[b] of shape (L, D)):
  scalar path: M = X @ w_l[c].T @ X.T + b_l[c]; s = sum_j tanh(M)_ij;  a = softmax(s)
  vector path: h = sigmoid(X @ w_v2[c] + b_v[c]); score = h @ w_v1[c]; av = softmax(score)
               pooled = av @ X
  out[b, l, d, c] = a[l] * X[l, d] + pooled[d]

Shapes: B=32, L=1024, D=768, C=2.  Sharding: batch across 8 cores (4 examples/core).
mm1/mm2 run in float32r (FP22) — the tanh-sum softmax amplifies operand noise
(s spread ~±100 acts like an argmax), so fp8 there fails the accuracy gate.
The vector-path G matmul tolerates fp8: it runs as fp8e4m3 DoubleRow (2 k-tiles
per instruction at 0.5 cyc/row), with X and w_v2 quantized on device.
"""
import numpy as np

try:
    import concourse.bass as bass
except ImportError:  # container staging path
    import sys
    sys.path.insert(0, "/opt/trn_rl_repo")
    import concourse.bass as bass
import concourse.tile as tile
from concourse import bacc, bass_isa, mybir
from concourse.bass_utils import run_bass_kernel_spmd
from contextlib import ExitStack

B, L, D, C = 32, 1024, 768, 2
NCORES = 8
BPC = B // NCORES          # examples per core
DT = D // 128              # 6 feature tiles
LTN = L // 128             # 8 sequence tiles
F32 = mybir.dt.float32
F32R = mybir.dt.float32r
FP8 = mybir.dt.float8e4
DR = mybir.MatmulPerfMode.DoubleRow
AF = mybir.ActivationFunctionType
AX = mybir.AxisListType
OP = mybir.AluOpType

SX = 32.0                  # fp8 scale for X
SWV = 1024.0               # fp8 scale for w_v2
GSCALE = 1.0 / (SX * SWV)  # dequant scale folded into the sigmoid
SWV1 = 1024.0              # fp8 scale for w_v1
# score path: sigmoid(g) - 0.5 = tanh(g/2)/2, and softmax is shift-invariant,
# so score' = 0.5 * (tanh((G+b_v)/2) @ w_v1) drops the constant 0.5*sum(w_v1).
# The centered tanh quantizes to fp8 with ~4x less absolute error than h.
ESCALE = 0.5 / SWV1        # dequant folded into the softmax exp

CFG = {
    "xn_bufs": 2, "h_bufs": 3, "ascr_bufs": 1, "ot_bufs": 4,
    "psmm_bufs": 3, "pstp_bufs": 2,
}
PHASE_MARKS = []


def _mark(nc, label):
    PHASE_MARKS.append((nc.next_id(), label))


def build_nc(bpc=BPC, repeat=1):
    nc = bacc.Bacc("TRN2", target_bir_lowering=False, debug=False)
    x_ext = nc.declare_dram_parameter("inputs", [bpc, L, D], F32R, isOutput=False)
    wl_ext = nc.declare_dram_parameter("w_l", [C, D, D], F32R, isOutput=False)
    bl_ext = nc.declare_dram_parameter("b_l", [C], F32, isOutput=False)
    wv1_ext = nc.declare_dram_parameter("w_v1", [C, D], F32R, isOutput=False)
    wv2_ext = nc.declare_dram_parameter("w_v2", [C, D, D], F32R, isOutput=False)
    bv_ext = nc.declare_dram_parameter("b_v", [C, D], F32, isOutput=False)
    out_ext = nc.declare_dram_parameter("out", [bpc, L, D, C], F32, isOutput=True)

    with tile.TileContext(nc) as tc:
        for _ in range(repeat):
            with ExitStack() as ctx:
                body(ctx, tc, x_ext.ap(), wl_ext.ap(), bl_ext.ap(), wv1_ext.ap(),
                     wv2_ext.ap(), bv_ext.ap(), out_ext.ap(), bpc)
    return nc


def body(ctx, tc, x_ext, wl_ext, bl_ext, wv1_ext, wv2_ext, bv_ext, out_ext, bpc=BPC):
    nc = tc.nc

    consts = ctx.enter_context(tc.tile_pool(name="consts", bufs=1))
    wtmp_pool = ctx.enter_context(tc.tile_pool(name="wtmp", bufs=2))
    xn_pool = ctx.enter_context(tc.tile_pool(name="xn", bufs=CFG["xn_bufs"]))
    xt_pool = ctx.enter_context(tc.tile_pool(name="xt", bufs=1))
    x8_pool = ctx.enter_context(tc.tile_pool(name="x8", bufs=1))
    ut_pool = ctx.enter_context(tc.tile_pool(name="ut", bufs=1))
    h_pool = ctx.enter_context(tc.tile_pool(name="h", bufs=CFG["h_bufs"]))
    a_pool = ctx.enter_context(tc.tile_pool(name="ascr", bufs=CFG["ascr_bufs"]))
    sm_pool = ctx.enter_context(tc.tile_pool(name="smx", bufs=2))
    pb_pool = ctx.enter_context(tc.tile_pool(name="pb", bufs=2))
    out_pool = ctx.enter_context(tc.tile_pool(name="ot", bufs=CFG["ot_bufs"]))
    vrow_pool = ctx.enter_context(tc.tile_pool(name="vrow", bufs=1))
    ps_mm = ctx.enter_context(tc.tile_pool(name="psmm", bufs=CFG["psmm_bufs"], space="PSUM"))
    ps_sv = ctx.enter_context(tc.tile_pool(name="pssv", bufs=2, space="PSUM"))

    # ---- constants / parameters ----
    # identity built on DVE (iota of n-p, then ==0) to keep startup short
    idx = consts.tile([128, 128], mybir.dt.int32)
    nc.gpsimd.iota(idx, pattern=[[1, 128]], base=0, channel_multiplier=-1)
    ident = consts.tile([128, 128], F32R)
    nc.vector.tensor_scalar(out=ident, in0=idx, scalar1=0, scalar2=None,
                            op0=OP.is_equal)


    # first example's X is DMA'd ahead of the weights so transposes start early
    xv = x_ext.rearrange("b (lt p) d -> b p lt d", p=128)
    xn0 = xn_pool.tile([128, LTN, D], F32R, tag="xn")
    for lt in range(LTN):
        nc.sync.dma_start(out=xn0[:, lt], in_=xv[0, :, lt])

    wl_sb = consts.tile([128, C, DT, D], F32R)      # [p, c, dt, e]: w_l[c, dt*128+p, e]
    wv28 = consts.tile([128, C, DT, D], FP8)        # fp8(w_v2 * SWV), same layout
    wlv = wl_ext.rearrange("c (dt p) e -> c p dt e", p=128)
    wv2v = wv2_ext.rearrange("c (et p) e -> c p et e", p=128)
    for et in range(DT):   # c0 weights in et-column order so mm1 starts early
        nc.sync.dma_start(out=wl_sb[:, 0, :, et * 128:(et + 1) * 128],
                          in_=wlv[0][:, :, et * 128:(et + 1) * 128])
    nc.sync.dma_start(out=wl_sb[:, 1], in_=wlv[1])
    # stage w_v2 through small transient tiles, quantizing to fp8 on DVE
    for c in range(C):
        for et in range(DT):
            wt = wtmp_pool.tile([128, D], F32R, tag="wt")
            nc.sync.dma_start(out=wt, in_=wv2v[c, :, et])
            nc.vector.tensor_scalar_mul(wv28[:, c, et], wt, SWV)
    wv1_sb = consts.tile([128, C, DT], F32R)        # [p, c, t] = w_v1[c, t*128+p]
    nc.sync.dma_start(out=wv1_sb, in_=wv1_ext.rearrange("c (t p) -> p c t", p=128))
    # fp8(w_v1 * SWV1) padded to stride 16 so the DoubleRow ifmap k-tile
    # stride is a multiple of 16 elements (ISA dual-fp8 restriction)
    wv18p = consts.tile([128, C, DT, 16], FP8)
    nc.scalar.memzero(wv18p)
    nc.vector.tensor_scalar_mul(wv18p[:, :, :, 0], wv1_sb, SWV1)
    bv_sb = consts.tile([128, C, DT], F32)          # [p, c, t] = b_v[c, t*128+p]
    nc.sync.dma_start(out=bv_sb, in_=bv_ext.rearrange("c (t p) -> p c t", p=128))
    bvh_sb = consts.tile([128, C, DT], F32)         # b_v / 2 for the tanh(g/2) form
    nc.vector.tensor_scalar_mul(bvh_sb, bv_sb, 0.5)
    bl_sb = consts.tile([128, C], F32)
    nc.sync.dma_start(out=bl_sb, in_=bl_ext.partition_broadcast(128))

    def emit_output(bo, xn_o, a8_o, pb_o):
        # out[l, d, c] = a[c][l] * X[l, d] + pooled[c][d], fused on DVE.
        # Emitted AFTER the next b's transpose drains so this long DVE chain
        # doesn't sit ahead of them in the DVE queue.
        _mark(nc, f'b{bo}:output')
        outv = out_ext[bo].rearrange("(lt p) d c -> lt p (d c)", p=128)
        HD = D // 2
        for lt in range(LTN):
            for dh in range(2):
                ot = out_pool.tile([128, HD * C], F32, tag="ot")
                otv = ot.rearrange("p (d c) -> p d c", c=2)
                for c in range(C):
                    nc.vector.scalar_tensor_tensor(
                        out=otv[:, :, c],
                        in0=xn_o[:, lt, dh * HD:(dh + 1) * HD],
                        scalar=a8_o[:, c, lt:lt + 1],
                        in1=pb_o[:, c, dh * HD:(dh + 1) * HD],
                        op0=OP.mult, op1=OP.add)
                nc.sync.dma_start(out=outv[lt][:, dh * HD * C:(dh + 1) * HD * C],
                                  in_=ot)

    xn_next = xn0
    pending = None
    for b in range(bpc):
        # ---- load X and build X^T (f32r) plus fp8 copy for the G matmul ----
        xn = xn_next
        _mark(nc, f'b{b}:transpose')
        xt = xt_pool.tile([128, DT, L], F32R)       # [p, dt, l] = X[l, dt*128+p]
        xh8 = x8_pool.tile([128, DT, L], FP8)       # fp8(X^T * SX)
        for dt in range(DT):
            tp = ps_mm.tile([128, 1024], F32R, tag="mm")
            for lt in range(LTN):
                nc.tensor.transpose(tp[:, lt * 128:(lt + 1) * 128],
                                    xn[:, lt, dt * 128:(dt + 1) * 128], ident)
            nc.scalar.copy(xt[:, dt, :], tp)
            nc.vector.tensor_scalar_mul(xh8[:, dt, :], tp, SX)
        if pending is not None:
            emit_output(*pending)
            pending = None
        if b + 1 < bpc:
            # prefetch the next example. Emitted AFTER emit_output so the
            # b-1 output reads of the buffer this overwrites are tracked, and
            # on the Activation HWDGE queue so it isn't serialized behind the
            # 6MB of output stores on the SP queue.
            xn_next = xn_pool.tile([128, LTN, D], F32R, tag="xn")
            nc.scalar.dma_start(out=xn_next, in_=xv[b + 1])

        a8_b = sm_pool.tile([128, C, LTN], F32, tag="a8b")   # softmax coeffs per c
        pb_b = pb_pool.tile([128, C, D], F32, tag="pbb")     # pooled broadcast per c

        if True:
            def phase_mm1(c):
                # scalar path: U^T[e, i] = sum_d w_l[c][d, e] * X^T[d, i]
                _mark(nc, f'b{b}c{c}:mm1')
                ut = ut_pool.tile([128, DT, L], F32R)
                for et in range(DT):
                    pm = ps_mm.tile([128, 1024], F32, tag="mm")
                    for ih in range(2):
                        for dt in range(DT):
                            nc.tensor.matmul(
                                pm[:, ih * 512:(ih + 1) * 512],
                                wl_sb[:, c, dt, et * 128:(et + 1) * 128],
                                xt[:, dt, ih * 512:(ih + 1) * 512],
                                start=(dt == 0), stop=(dt == DT - 1),
                                skip_group_check=True)
                    nc.vector.tensor_copy(ut[:, et, :], pm)
                return ut

            def phase_vec(c):
                # vector path (transposed): G^T[e2, l] = w_v2[c]^T X^T in fp8
                # DoubleRow. h' = tanh((G+b_v)/2) = sigmoid(G+b_v)-0.5 stored
                # as fp8 (softmax is shift-invariant, the centered value
                # quantizes ~4x better than h). The score matmul then runs
                # fp8 DoubleRow with hh2 as the 128-col stationary, landing
                # score^T directly in a [128, LTN] psum — no row transpose.
                _mark(nc, f'b{b}c{c}:vecG')
                hh2s = []
                for et2p in range(0, DT, 2):
                    hh2 = h_pool.tile([128, 2, 1024], FP8, tag="h")
                    for j in range(2):
                        et2 = et2p + j
                        pg = ps_mm.tile([128, 1024], F32, tag="mm")
                        for lh in range(2):
                            for dt3 in range(0, DT, 2):
                                nc.tensor.matmul(
                                    pg[:, lh * 512:(lh + 1) * 512],
                                    wv28[:, c, dt3:dt3 + 2, et2 * 128:(et2 + 1) * 128],
                                    xh8[:, dt3:dt3 + 2, lh * 512:(lh + 1) * 512],
                                    start=(dt3 == 0), stop=(dt3 == DT - 2),
                                    perf_mode=DR, skip_group_check=True)
                        nc.scalar.activation(out=hh2[:, j], in_=pg, func=AF.Tanh,
                                             scale=GSCALE * 0.5,
                                             bias=bvh_sb[:, c, et2:et2 + 1])
                    hh2s.append(hh2)
                pv = ps_sv.tile([128, LTN], F32, tag="sv")
                for k in range(LTN):
                    for pi, et2p in enumerate(range(0, DT, 2)):
                        nc.tensor.matmul(
                            pv[:, k:k + 1],
                            hh2s[pi][:, :, k * 128:(k + 1) * 128],
                            wv18p[:, c, et2p:et2p + 2, 0:1],
                            start=(et2p == 0), stop=(et2p == DT - 2),
                            perf_mode=DR, skip_group_check=True)
                sv8 = sm_pool.tile([128, LTN], F32, tag="s8")
                nc.vector.tensor_copy(sv8, pv)
                _mark(nc, f'b{b}c{c}:softmaxV')
                # scores are O(1): exp cannot overflow, so skip the
                # max-subtraction; ESCALE dequantizes the fp8 score here
                ev8 = sm_pool.tile([128, LTN], F32, tag="e8")
                vesum = sm_pool.tile([128, 1], F32, tag="sm4")
                nc.scalar.activation(out=ev8, in_=sv8, func=AF.Exp,
                                     scale=ESCALE, accum_out=vesum)
                vgsum = sm_pool.tile([128, 1], F32, tag="sm5")
                nc.gpsimd.partition_all_reduce(vgsum, vesum, 128,
                                               bass_isa.ReduceOp.add)
                vrz = sm_pool.tile([128, 1], F32, tag="sm6")
                nc.vector.reciprocal(vrz, vgsum)
                avt = sm_pool.tile([128, LTN], F32R, tag="avt")
                nc.vector.tensor_scalar_mul(avt, ev8, vrz)
                return avt

            def phase_mm2(ut, c):
                # M tiles + fused tanh/row-sum -> s8 columns
                _mark(nc, f'b{b}c{c}:mm2')
                s8 = sm_pool.tile([128, 8], F32, tag="s8")
                for it in range(LTN):
                    pm = ps_mm.tile([128, 1024], F32, tag="mm")
                    for jh in range(2):
                        for et in range(DT):
                            nc.tensor.matmul(
                                pm[:, jh * 512:(jh + 1) * 512],
                                ut[:, et, it * 128:(it + 1) * 128],
                                xt[:, et, jh * 512:(jh + 1) * 512],
                                start=(et == 0), stop=(et == DT - 1),
                                skip_group_check=True)
                    ascr = a_pool.tile([128, 1024], F32, tag="ascr")
                    nc.scalar.activation(
                        out=ascr, in_=pm, func=AF.Tanh,
                        bias=bl_sb[:, c:c + 1], scale=1.0,
                        accum_out=s8[:, it:it + 1])
                return s8

            def phase_pooled(avt, c):
                # pooled = sum_l av[l] X[l, :], in two single-bank slots
                _mark(nc, f'b{b}c{c}:pooled')
                pla = ps_sv.tile([1, 512], F32, tag="sv")
                plb = ps_sv.tile([1, 512], F32, tag="sv")
                for k in range(LTN):
                    nc.tensor.matmul(pla[0:1, 0:512], avt[:, k:k + 1],
                                     xn[:, k, 0:512],
                                     start=(k == 0), stop=(k == LTN - 1),
                                     skip_group_check=True)
                    nc.tensor.matmul(plb[0:1, 0:256], avt[:, k:k + 1],
                                     xn[:, k, 512:768],
                                     start=(k == 0), stop=(k == LTN - 1),
                                     skip_group_check=True)
                pooled_sb = vrow_pool.tile([1, D], F32, tag="psb")
                nc.vector.tensor_copy(pooled_sb[:, 0:512], pla[0:1, :])
                nc.vector.tensor_copy(pooled_sb[:, 512:768], plb[0:1, 0:256])
                nc.gpsimd.partition_broadcast(pb_b[:, c, :], pooled_sb, 128)

            def phase_smaxS(s8, c):
                # softmax over all 1024 positions (partitions x tiles)
                _mark(nc, f'b{b}c{c}:softmaxS')
                rmax = sm_pool.tile([128, 1], F32, tag="sm1")
                nc.vector.reduce_max(out=rmax, in_=s8, axis=AX.X)
                gmax = sm_pool.tile([128, 1], F32, tag="sm2")
                nc.gpsimd.partition_all_reduce(gmax, rmax, 128,
                                               bass_isa.ReduceOp.max)
                negm = sm_pool.tile([128, 1], F32, tag="sm3")
                nc.vector.tensor_scalar_mul(negm, gmax, -1.0)
                e8 = sm_pool.tile([128, 8], F32, tag="e8")
                esum = sm_pool.tile([128, 1], F32, tag="sm4")
                nc.scalar.activation(out=e8, in_=s8, func=AF.Exp, bias=negm,
                                     scale=1.0, accum_out=esum)
                gsum = sm_pool.tile([128, 1], F32, tag="sm5")
                nc.gpsimd.partition_all_reduce(gsum, esum, 128,
                                               bass_isa.ReduceOp.add)
                rz = sm_pool.tile([128, 1], F32, tag="sm6")
                nc.vector.reciprocal(rz, gsum)
                nc.vector.tensor_scalar_mul(a8_b[:, c, :], e8, rz)

            # pooled runs once for both channels after mm2_1 so the softmaxV
            # chains (DVE/Act/Pool hops) complete in the shadow of PE matmul
            # work instead of stalling the PE right after the score matmuls
            ut0 = phase_mm1(0)
            avt0 = phase_vec(0)
            s80 = phase_mm2(ut0, 0)
            ut1 = phase_mm1(1)
            avt1 = phase_vec(1)
            phase_pooled(avt0, 0)
            s81 = phase_mm2(ut1, 1)
            phase_smaxS(s80, 0)
            phase_pooled(avt1, 1)
            phase_smaxS(s81, 1)

        pending = (b, xn, a8_b, pb_b)

    emit_output(*pending)


_NC_CACHE = {}


def _get_nc():
    if "nc" not in _NC_CACHE:
        nc = build_nc()
        nc.compile()
        _NC_CACHE["nc"] = nc
    return _NC_CACHE["nc"]


def kernel(inputs, w_l, b_l, w_v1, w_v2, b_v):
    nc = _get_nc()
    inputs = np.ascontiguousarray(np.asarray(inputs, np.float32))
    shared = {
        "w_l": np.ascontiguousarray(np.asarray(w_l, np.float32)),
        "b_l": np.ascontiguousarray(np.asarray(b_l, np.float32)),
        "w_v1": np.ascontiguousarray(np.asarray(w_v1, np.float32)),
        "w_v2": np.ascontiguousarray(np.asarray(w_v2, np.float32)),
        "b_v": np.ascontiguousarray(np.asarray(b_v, np.float32)),
    }
    in_maps = [
        {"inputs": inputs[i * BPC:(i + 1) * BPC], **shared} for i in range(NCORES)
    ]
    res = run_bass_kernel_spmd(nc, in_maps, core_ids=list(range(NCORES)))
    return np.concatenate([res.results[i]["out"] for i in range(NCORES)], axis=0)


if __name__ == "__main__":
    rng = np.random.default_rng(0)
    out = kernel(
        inputs=rng.standard_normal((B, L, D)).astype(np.float32),
        w_l=0.02 * rng.standard_normal((C, D, D)).astype(np.float32),
        b_l=0.02 * rng.standard_normal((C,)).astype(np.float32),
        w_v1=0.02 * rng.standard_normal((C, D)).astype(np.float32),
        w_v2=0.02 * rng.standard_normal((C, D, D)).astype(np.float32),
        b_v=0.02 * rng.standard_normal((C, D)).astype(np.float32),
    )
    print("out", out.shape, out.dtype, np.abs(out).mean())
